# revision 23
# baseline (speedup 1.0000x reference)
"""Trainium2 Bass kernel for nn_EventSequenceDurationGraphConvModel.

Self-contained: accepts FULL inputs, shards across 8 NeuronCores internally
(nodes/edges partitioned by destination node per core), runs one SPMD Bass
program, and returns the FULL [64, 16] output.

Key design points (v2):
  * Selection matrices (wsel[e, d] = ew[e] * (dst_rel[e] == d)) are built on
    the HOST and streamed from HBM -- the v1 kernel built them on DVE, which
    saturated the vector engine AND stalled GPSIMD descriptor generation via
    SBUF port contention.
  * g1/d1 need no device gathers at all: the host pre-gathers x[src] /
    dur_x[src] per edge slot into matmul-ready chunk planes.
  * g2 gathers a PRE-PROJECTED table h1p = h1 @ g2_Wr, so the chunk matmuls
    accumulate output features directly (root term + bias fold into the same
    PSUM chain; no aggT round-trip).
  * c1 gathers ONE interleaved pre-projected table hdp = [h2|d] @ c1_Wr
    (512-byte rows, half the descriptor count of two separate tables).
  * Feature-major dataflow: layer outputs stay [feat, node] in SBUF for the
    next layer's root term; node-major copies for the gather tables are
    produced by the projection matmuls themselves (lhsT = featT slice).
  * Pooling one-hot has 1/graph_count baked in on host; AllReduce of the
    pooled [64, 256] then a replicated MLP head + log_softmax.
"""
import sys

import numpy as np

sys.path.insert(0, "/opt/trn_rl_repo")

from concourse import bacc, bass, mybir  # noqa: E402
import concourse.tile as tile  # noqa: E402
from concourse.masks import make_identity  # noqa: E402

F32 = mybir.dt.float32
BF16 = mybir.dt.bfloat16
I16 = mybir.dt.int16
AF = mybir.ActivationFunctionType
OP = mybir.AluOpType

NC = 8
GP = 2          # blocks per gather group (index-plane granularity)
NBLK1 = 24      # blocks in gather-table half 1 (locals < NBLK1*128)
REAL = dict(N=50000, E=800000, B=64, SHARD=6250, SHARD_PAD=6400)


# --------------------------------------------------------------------------
# Host-side preprocessing
# --------------------------------------------------------------------------

def _gpid(node_id, cfg):
    """Real node id -> padded global id."""
    return (node_id // cfg["SHARD"]) * cfg["SHARD_PAD"] + node_id % cfg["SHARD"]


def _wrap_idx(flat_i16):
    """Flat int16 index list -> dma_gather plane [128, n/16] (16-part wrap,
    replicated across the 8 gpsimd cores)."""
    n = flat_i16.shape[0]
    assert n % 16 == 0
    return np.tile(flat_i16.reshape(n // 16, 16).T, (8, 1)).copy()


def prep_edges(edge_index, edge_attr, gtable_bf16, fin, cfg,
               min_ca=1, min_cb=1):
    """Shard + chunk the edge list; build host-side wsel planes, pre-gathered
    source-row planes, and int16 gather index planes.

    Chunk order (shared by all planes): group-major over gather groups of GP
    blocks; within a group all A-half chunks (block-major), then all B-half
    chunks.  Slot (chunk c, partition p) = edge c*128 + p.

    Returns per-core dicts and (CA, CB).
    """
    import ml_dtypes

    n_blk = cfg["SHARD_PAD"] // 128
    L1 = NBLK1 * 128                    # locals below L1 live in table 1
    L2 = cfg["SHARD_PAD"] - L1
    src = np.asarray(edge_index[0], dtype=np.int64)
    dst = np.asarray(edge_index[1], dtype=np.int64)
    ew = np.asarray(edge_attr, dtype=np.float32)
    gsrc = _gpid(src, cfg)
    sshard = src // cfg["SHARD"]
    sloc_pad = gsrc % cfg["SHARD_PAD"]
    in_t1 = sloc_pad < L1
    # row index within gather table 1 / 2
    trow = np.where(in_t1, sshard * L1 + sloc_pad,
                    sshard * L2 + (sloc_pad - L1))
    core = dst // cfg["SHARD"]
    dloc = dst % cfg["SHARD"]

    # bucket edges per (core, block, table-half)
    buckets = [[None] * n_blk for _ in range(NC)]
    ca_max, cb_max = min_ca, min_cb
    for k in range(NC):
        sel = np.nonzero(core == k)[0]
        dl = dloc[sel]
        blk = dl // 128
        order = np.argsort(blk, kind="stable")
        sel, dl, blk = sel[order], dl[order], blk[order]
        bnd = np.searchsorted(blk, np.arange(n_blk + 1))
        for b in range(n_blk):
            s = sel[bnd[b]:bnd[b + 1]]
            ina = in_t1[s]
            a_idx, b_idx = s[ina], s[~ina]
            buckets[k][b] = (a_idx, b_idx)
            ca_max = max(ca_max, -(-len(a_idx) // 128))
            cb_max = max(cb_max, -(-len(b_idx) // 128))

    CA, CB = ca_max, cb_max
    C = CA + CB
    n_grp = -(-n_blk // GP)
    assert n_blk % GP == 0, "pad SHARD_PAD so blocks divide into GP groups"
    totch = n_blk * C  # total chunks per core

    out = []
    for k in range(NC):
        # slot arrays over all chunks
        eslot = np.full(totch * 128, -1, np.int64)   # edge id or -1
        for g in range(n_grp):
            base = g * GP * C * 128
            for r in range(GP):
                b = g * GP + r
                a_idx, b_idx = buckets[k][b]
                oa = base + (r * CA) * 128
                eslot[oa:oa + len(a_idx)] = a_idx
                ob = base + (GP * CA + r * CB) * 128
                eslot[ob:ob + len(b_idx)] = b_idx
        valid = eslot >= 0
        e_safe = np.where(valid, eslot, 0)

        # wsel plane [128, totch*128] bf16
        relv = np.where(valid, dloc[e_safe] % 128, 0).astype(np.int64)
        wv = np.where(valid, ew[e_safe], 0.0).astype(np.float32)
        cidx = np.arange(totch * 128) // 128
        pidx = np.arange(totch * 128) % 128
        wplane = np.zeros((128, totch * 128), np.float32)
        wplane[pidx, cidx * 128 + relv] = wv
        wplane = wplane.astype(ml_dtypes.bfloat16)

        # pre-gathered source rows [128, totch*fin] bf16
        rows = gtable_bf16[np.where(valid, gsrc[e_safe], 0)]
        rows[~valid] = 0
        xg = np.ascontiguousarray(
            rows.reshape(totch, 128, fin).transpose(1, 0, 2).reshape(
                128, totch * fin))

        # int16 index planes (A first, then B, group-major)
        ga = np.where(valid, trow[e_safe], 0)
        flat_a = np.zeros(n_blk * CA * 128, np.int16)
        flat_b = np.zeros(n_blk * CB * 128, np.int16)
        for g in range(n_grp):
            base = g * GP * C * 128
            na, nb = GP * CA * 128, GP * CB * 128
            flat_a[g * na:(g + 1) * na] = ga[base:base + na]
            flat_b[g * nb:(g + 1) * nb] = np.where(
                valid[base + na:base + na + nb],
                ga[base + na:base + na + nb], 0)
        out.append(dict(
            wsel=wplane, xg=xg,
            idx_a=_wrap_idx(flat_a), idx_b=_wrap_idx(flat_b),
        ))
    return out, CA, CB


def _pad_nodes(arr, cfg):
    """[N, F] -> [NC*SHARD_PAD, F] with zero-filled pad rows per shard."""
    f = arr.shape[1]
    out = np.zeros((NC * cfg["SHARD_PAD"], f), arr.dtype)
    for k in range(NC):
        out[k * cfg["SHARD_PAD"]:k * cfg["SHARD_PAD"] + cfg["SHARD"]] = (
            arr[k * cfg["SHARD"]:(k + 1) * cfg["SHARD"]]
        )
    return out


# --------------------------------------------------------------------------
# Device program
# --------------------------------------------------------------------------

def build_program(cfg, CA, CB, CDA, CDB):
    import os as _os
    n_blk = cfg["SHARD_PAD"] // 128
    npad = NC * cfg["SHARD_PAD"]
    half = npad // 2
    B = cfg["B"]
    C = CA + CB
    CD = CDA + CDB
    n_grp = n_blk // GP
    kphase = int(_os.environ.get("KPHASE", "9"))

    nc = bacc.Bacc("TRN2", target_bir_lowering=False, debug=False,
                   num_devices=NC)

    def din(name, shape, dt=F32):
        return nc.declare_dram_parameter(name, list(shape), dt, isOutput=False)

    # edge planes
    ev_wsel = din("ev_wsel", [128, n_blk * C * 128], BF16)
    ev_xg = din("ev_xg", [128, n_blk * C * 128], BF16)
    ev_idx_a = din("ev_idx_a", [128, n_blk * CA * 8], I16)
    ev_idx_b = din("ev_idx_b", [128, n_blk * CB * 8], I16)
    du_wsel = din("du_wsel", [128, n_blk * CD * 128], BF16)
    du_xg = din("du_xg", [128, n_blk * CD * 64], BF16)
    # node-feature planes (feature-major, bf16)
    xT_in = din("xT", [128, cfg["SHARD_PAD"]], BF16)
    dxT_in = din("dxT", [64, cfg["SHARD_PAD"]], BF16)
    ssel_in = din("ssel", [128, n_blk * B], BF16)
    seq_in = din("seq_features", [B, 256])

    wnames = [
        ("g1_Wr", [128, 128], BF16), ("g1_Wroot", [128, 128], BF16),
        ("g1_br", [1, 128], BF16),
        ("g2_Wr", [128, 128], BF16), ("g2_Wroot", [128, 128], BF16),
        ("g2_br", [1, 128], BF16),
        ("d1_Wr", [64, 128], BF16), ("d1_Wroot", [64, 128], BF16),
        ("d1_br", [1, 128], BF16),
        ("c1_Wr", [256, 256], BF16),   # 2 tiles [128, 256]
        ("w_rs", [256, 256], BF16),    # c1_Wroot + skip_W
        ("bias_c", [1, 256], BF16),
        ("fc1_W", [256, 256], F32), ("fc1_b", [256, 1], F32),
        ("fc2_W", [256, 128], F32), ("fc2_b", [128, 1], F32),
        ("fcc_W", [384, 256], F32), ("fcc_b", [256, 1], F32),
        ("cls_W", [256, 16], F32), ("cls_b_rep", [B, 16], F32),
    ]
    wdram = {nm: din(nm, sh, dt) for nm, sh, dt in wnames}
    out_ext = nc.declare_dram_parameter("out", [B, 16], F32, isOutput=True)

    from contextlib import ExitStack
    with tile.TileContext(nc) as tc, ExitStack() as ctx:
        cpool = ctx.enter_context(tc.tile_pool(name="const", bufs=1))
        spool = ctx.enter_context(tc.tile_pool(name="sbuf", bufs=3))
        wpool = ctx.enter_context(tc.tile_pool(name="wstream", bufs=2))
        xpool = ctx.enter_context(tc.tile_pool(name="xstream", bufs=3))
        pagg = ctx.enter_context(tc.tile_pool(name="pagg", bufs=2,
                                              space="PSUM"))
        pagg2 = ctx.enter_context(tc.tile_pool(name="pagg2", bufs=2,
                                               space="PSUM"))
        pproj = ctx.enter_context(tc.tile_pool(name="pproj", bufs=2,
                                               space="PSUM"))
        ppjhd = ctx.enter_context(tc.tile_pool(name="ppjhd", bufs=1,
                                               space="PSUM"))
        pacc = ctx.enter_context(tc.tile_pool(name="pacc", bufs=1,
                                              space="PSUM"))
        dpool = ctx.enter_context(tc.tile_pool(name="dram", bufs=1,
                                               space="DRAM"))

        # ---- constants ----------------------------------------------------
        ident = cpool.tile([128, 128], F32, tag="ident")
        make_identity(nc, ident[:])
        ones_row = cpool.tile([1, 128], BF16, tag="ones_row")
        nc.vector.memset(ones_row[:], 1.0)

        def wtiles(nm, rows, cols, dt):
            ts = []
            for i in range(0, rows, 128):
                p = min(128, rows - i)
                t = cpool.tile([p, cols], dt, tag=f"w_{nm}_{i}")
                nc.sync.dma_start(out=t[:], in_=wdram[nm][i:i + p, :])
                ts.append(t)
            return ts

        W = {nm: wtiles(nm, sh[0], sh[1], dt) for nm, sh, dt in wnames}

        ev_ia = cpool.tile([128, n_blk * CA * 8], I16, tag="ev_ia")
        nc.sync.dma_start(out=ev_ia[:], in_=ev_idx_a[:])
        ev_ib = cpool.tile([128, n_blk * CB * 8], I16, tag="ev_ib")
        nc.sync.dma_start(out=ev_ib[:], in_=ev_idx_b[:])
        xT = cpool.tile([128, cfg["SHARD_PAD"]], BF16, tag="xT")
        nc.sync.dma_start(out=xT[:], in_=xT_in[:])
        dxT = cpool.tile([64, cfg["SHARD_PAD"]], BF16, tag="dxT")
        nc.sync.dma_start(out=dxT[:], in_=dxT_in[:])
        ssel = cpool.tile([128, n_blk * B], BF16, tag="ssel")
        nc.sync.dma_start(out=ssel[:], in_=ssel_in[:])

        # resident feature-major layer outputs
        h1T = cpool.tile([128, cfg["SHARD_PAD"]], BF16, tag="h1T")
        h2T = cpool.tile([128, cfg["SHARD_PAD"]], BF16, tag="h2T")
        dT = cpool.tile([128, cfg["SHARD_PAD"]], BF16, tag="dT")

        # ---- DRAM intermediates -------------------------------------------
        # gather tables are split by block-half so each AllGather can fire
        # as soon as its blocks are done (hides collective latency)
        L1 = NBLK1 * 128
        L2 = cfg["SHARD_PAD"] - L1
        h1p_loc1 = dpool.tile([L1, 128], BF16, tag="h1p_loc1")
        h1p_loc2 = dpool.tile([L2, 128], BF16, tag="h1p_loc2")
        h1p_t1 = dpool.tile([NC * L1, 128], BF16, tag="h1p_t1",
                            addr_space="Shared")
        h1p_t2 = dpool.tile([NC * L2, 128], BF16, tag="h1p_t2",
                            addr_space="Shared")
        hdp_loc1 = dpool.tile([L1, 256], BF16, tag="hdp_loc1")
        hdp_loc2 = dpool.tile([L2, 256], BF16, tag="hdp_loc2")
        hdp_t1 = dpool.tile([NC * L1, 256], BF16, tag="hdp_t1",
                            addr_space="Shared")
        hdp_t2 = dpool.tile([NC * L2, 256], BF16, tag="hdp_t2",
                            addr_space="Shared")
        h1p_loc = [h1p_loc1, h1p_loc2]
        h1p_t = [h1p_t1, h1p_t2]
        hdp_loc = [hdp_loc1, hdp_loc2]
        hdp_t = [hdp_t1, hdp_t2]
        ar_in = dpool.tile([B, 256], F32, tag="ar_in")
        ar_out = dpool.tile([B, 256], F32, tag="ar_out", addr_space="Shared")

        def locrows(b):
            """(tile_index, row_start) for block b in a split local table."""
            if b < NBLK1:
                return 0, b * 128
            return 1, (b - NBLK1) * 128

        def bias_mm(ps, brow, cols=128, first=False, last=False):
            """out[o, d] += br[o] x 1[d] via 1-partition outer product."""
            nc.tensor.matmul(out=ps, lhsT=brow, rhs=ones_row[:, :cols],
                             start=first, stop=last)

        # ---- phase A: g1 (plain) + d1 (plain) -----------------------------
        # Stream granularity = one gather group (GP blocks); g2 gathers span
        # GSG = 2*GP blocks to amortize Q7 fixed cost; c1 gathers span GP
        # blocks (512B rows -> same gath tile shape as g2).
        n_grp2 = n_blk // GP

        def stream_tiles(g, plane, cw, pool, tag):
            t = pool.tile([128, GP * cw], BF16, tag=tag)
            nc.sync.dma_start(
                out=t[:], in_=plane[:, g * GP * cw:(g + 1) * GP * cw])
            return t

        def cpos_f(j, r):
            if j < CA:
                return r * CA + j
            return GP * CA + r * CB + (j - CA)

        def cposd_f(j, r):
            if j < CDA:
                return r * CDA + j
            return GP * CDA + r * CDB + (j - CDA)

        # --- g1 ---
        if kphase >= 1:
            for g in range(n_grp2):
                wt = stream_tiles(g, ev_wsel, C * 128, wpool, "wsel_ev")
                xt = xpool.tile([128, 2 * GP * C * 128], BF16, tag="gx")
                nc.sync.dma_start(
                    out=xt[:, :GP * C * 128],
                    in_=ev_xg[:, g * GP * C * 128:(g + 1) * GP * C * 128])
                for r in range(GP):
                    b = g * GP + r
                    agg = pagg.tile([128, 128], F32, tag="agg")
                    for j in range(C):
                        cp = cpos_f(j, r)
                        nc.tensor.matmul(
                            out=agg[:], lhsT=xt[:, cp * 128:(cp + 1) * 128],
                            rhs=wt[:, cp * 128:(cp + 1) * 128],
                            start=(j == 0), stop=(j == C - 1))
                    aggs = spool.tile([128, 128], BF16, tag="agg_sb")
                    nc.scalar.copy(aggs[:], agg[:])
                    st = pagg2.tile([128, 128], F32, tag="st")
                    nc.tensor.matmul(out=st[:], lhsT=W["g1_Wr"][0][:],
                                     rhs=aggs[:], start=True, stop=False)
                    nc.tensor.matmul(out=st[:], lhsT=W["g1_Wroot"][0][:],
                                     rhs=xT[:, b * 128:(b + 1) * 128],
                                     start=False, stop=False)
                    bias_mm(st[:], W["g1_br"][0][:], last=True)
                    nc.scalar.activation(out=h1T[:, b * 128:(b + 1) * 128],
                                         in_=st[:], func=AF.Relu)
                    # h1p block (node-major, projected by g2_Wr)
                    pj = pproj.tile([128, 128], F32, tag="pj1")
                    nc.tensor.matmul(out=pj[:],
                                     lhsT=h1T[:, b * 128:(b + 1) * 128],
                                     rhs=W["g2_Wr"][0][:],
                                     start=True, stop=True)
                    pjs = spool.tile([128, 128], BF16, tag="pj1_sb")
                    nc.scalar.copy(pjs[:], pj[:])
                    ti, row = locrows(b)
                    nc.sync.dma_start(
                        out=h1p_loc[ti][row:row + 128, :], in_=pjs[:])
                if kphase >= 2 and g == NBLK1 // GP - 1:
                    nc.gpsimd.collective_compute(
                        "AllGather", OP.bypass,
                        replica_groups=[list(range(NC))],
                        ins=[h1p_loc[0].opt()], outs=[h1p_t[0].opt()])

        if kphase >= 2:
            nc.gpsimd.collective_compute(
                "AllGather", OP.bypass, replica_groups=[list(range(NC))],
                ins=[h1p_loc[1].opt()], outs=[h1p_t[1].opt()])

        # --- d1 (emitted interleaved into phase B: its PE work overlaps the
        #     Q7-bound g2 gathers without delaying g1 -> AG(h1p)) ---
        def emit_d1_group(g):
                wt = stream_tiles(g, du_wsel, CD * 128, wpool, "wsel_ev")
                xt = stream_tiles(g, du_xg, CD * 64, xpool, "xg_d")
                for r in range(GP):
                    b = g * GP + r
                    agg = pagg.tile([64, 128], F32, tag="agg")
                    for j in range(CD):
                        cp = cposd_f(j, r)
                        nc.tensor.matmul(
                            out=agg[:], lhsT=xt[:, cp * 64:(cp + 1) * 64],
                            rhs=wt[:, cp * 128:(cp + 1) * 128],
                            start=(j == 0), stop=(j == CD - 1))
                    aggs = spool.tile([64, 128], BF16, tag="agg_sb")
                    nc.scalar.copy(aggs[:], agg[:])
                    st = pagg2.tile([128, 128], F32, tag="st")
                    nc.tensor.matmul(out=st[:], lhsT=W["d1_Wr"][0][:],
                                     rhs=aggs[:], start=True, stop=False)
                    nc.tensor.matmul(out=st[:], lhsT=W["d1_Wroot"][0][:],
                                     rhs=dxT[:, b * 128:(b + 1) * 128],
                                     start=False, stop=False)
                    bias_mm(st[:], W["d1_br"][0][:], last=True)
                    nc.scalar.activation(out=dT[:, b * 128:(b + 1) * 128],
                                         in_=st[:], func=AF.Relu)
                    # hdp d-half contribution: dT^T @ c1_Wr[128:256, :]
                    pj = ppjhd.tile([128, 256], F32, tag="pjhd")
                    nc.tensor.matmul(out=pj[:],
                                     lhsT=dT[:, b * 128:(b + 1) * 128],
                                     rhs=W["c1_Wr"][1][:],
                                     start=True, stop=True)
                    pjs = spool.tile([128, 256], BF16, tag="pjhd_sb")
                    nc.scalar.copy(pjs[:], pj[:])
                    ti, row = locrows(b)
                    nc.sync.dma_start(
                        out=hdp_loc[ti][row:row + 128, :], in_=pjs[:])

        if kphase < 4:
            t = spool.tile([B, 16], F32, tag="early")
            nc.sync.dma_start(out=t[:], in_=h1p_loc[0][0:B, 0:16])
            nc.sync.dma_start(out=out_ext[:], in_=t[:])

        # --- phase B: g2 (gathers pre-projected h1p) -----------------------
        GSUP = 2  # gather groups per dma_gather pair
        if kphase >= 4:
            n_sup = -(-n_grp2 // GSUP)

            def nsub_of(gs):
                return min(GSUP, n_grp2 - gs * GSUP)

            def emit_gath_b(gs, gath, half_i):
                n_sub = nsub_of(gs)
                idx, cc, off, th = (
                    (ev_ia, CA, 0, h1p_t[0][:]) if half_i == 0 else
                    (ev_ib, CB, n_sub * GP * CA, h1p_t[1][:]))
                span = n_sub * GP * cc
                g0 = gs * GSUP
                nc.gpsimd.dma_gather(
                    out_ap=gath[:, off * 128:(off + span) * 128].rearrange(
                        "p (c f) -> p c f", c=span),
                    in_ap=th,
                    idxs_ap=idx[:, g0 * GP * cc * 8:
                                (g0 + n_sub) * GP * cc * 8],
                    num_idxs=span * 128,
                    num_idxs_reg=span * 128,
                    elem_size=128,
                    single_packet=False,
                )

            # prefetch two supergroups of table-1 gathers across the AG2 wait
            g_tiles = {}
            for pre in range(min(2, n_sup)):
                gt = xpool.tile([128, GSUP * GP * C * 128], BF16, tag="gx")
                g_tiles[pre] = gt
                emit_gath_b(pre, gt, 0)
            for gs in range(n_sup):
                n_sub = nsub_of(gs)
                gath = g_tiles.pop(gs)
                emit_gath_b(gs, gath, 1)
                if gs + 2 < n_sup:
                    gt = xpool.tile([128, GSUP * GP * C * 128], BF16,
                                    tag="gx")
                    g_tiles[gs + 2] = gt
                    emit_gath_b(gs + 2, gt, 0)
                for dg in range(gs * GSUP, min((gs + 1) * GSUP, n_grp2)):
                    emit_d1_group(dg)
                for sub in range(n_sub):
                    g = gs * GSUP + sub
                    wt = stream_tiles(g, ev_wsel, C * 128, wpool, "wsel_ev")
                    for r in range(GP):
                        b = g * GP + r

                        def gpos(j, sub=sub, r=r, n_sub=n_sub):
                            if j < CA:
                                return (sub * GP + r) * CA + j
                            return (n_sub * GP * CA + (sub * GP + r) * CB
                                    + (j - CA))

                        st = pagg.tile([128, 128], F32, tag="agg")
                        for j in range(C):
                            gp_, cp = gpos(j), cpos_f(j, r)
                            nc.tensor.matmul(
                                out=st[:],
                                lhsT=gath[:, gp_ * 128:(gp_ + 1) * 128],
                                rhs=wt[:, cp * 128:(cp + 1) * 128],
                                start=(j == 0), stop=False)
                        nc.tensor.matmul(out=st[:], lhsT=W["g2_Wroot"][0][:],
                                         rhs=h1T[:, b * 128:(b + 1) * 128],
                                         start=False, stop=False)
                        bias_mm(st[:], W["g2_br"][0][:], last=True)
                        nc.scalar.activation(
                            out=h2T[:, b * 128:(b + 1) * 128],
                            in_=st[:], func=AF.Relu)
                        # hdp h2-half contribution (+ d-half already in DRAM)
                        pj = ppjhd.tile([128, 256], F32, tag="pjhd")
                        nc.tensor.matmul(out=pj[:],
                                         lhsT=h2T[:, b * 128:(b + 1) * 128],
                                         rhs=W["c1_Wr"][0][:],
                                         start=True, stop=True)
                        prev = spool.tile([128, 256], BF16, tag="hd_prev")
                        ti, row = locrows(b)
                        nc.sync.dma_start(
                            out=prev[:], in_=hdp_loc[ti][row:row + 128, :])
                        pjs = spool.tile([128, 256], BF16, tag="pjhd_sb")
                        nc.vector.tensor_tensor(out=pjs[:], in0=pj[:],
                                                in1=prev[:], op=OP.add)
                        nc.sync.dma_start(
                            out=hdp_loc[ti][row:row + 128, :], in_=pjs[:])
                if kphase >= 5 and (gs + 1) * GSUP * GP == NBLK1:
                    nc.gpsimd.collective_compute(
                        "AllGather", OP.bypass,
                        replica_groups=[list(range(NC))],
                        ins=[hdp_loc[0].opt()], outs=[hdp_t[0].opt()])

        if kphase >= 5:
            nc.gpsimd.collective_compute(
                "AllGather", OP.bypass, replica_groups=[list(range(NC))],
                ins=[hdp_loc[1].opt()], outs=[hdp_t[1].opt()])

        if 4 <= kphase < 6:
            t = spool.tile([B, 16], F32, tag="early")
            nc.sync.dma_start(out=t[:], in_=hdp_loc[0][0:B, 0:16])
            nc.sync.dma_start(out=out_ext[:], in_=t[:])

        # --- phase C: c1 + pooling ----------------------------------------
        if kphase >= 6:
            pool_ps = pacc.tile([B, 256], F32, tag="pool")

            def emit_gath_c(g, gath, half_i):
                idx, cc, off, th = (
                    (ev_ia, CA, 0, hdp_t[0][:]) if half_i == 0 else
                    (ev_ib, CB, GP * CA, hdp_t[1][:]))
                span = GP * cc
                nc.gpsimd.dma_gather(
                    out_ap=gath[:, off * 256:(off + span) * 256].rearrange(
                        "p (c f) -> p c f", c=span),
                    in_ap=th,
                    idxs_ap=idx[:, g * GP * cc * 8:(g + 1) * GP * cc * 8],
                    num_idxs=span * 128,
                    num_idxs_reg=span * 128,
                    elem_size=256,
                    single_packet=False,
                )

            c_tiles = {}
            for pre in range(min(2, n_grp2)):
                ct = xpool.tile([128, GP * C * 256], BF16, tag="gx")
                c_tiles[pre] = ct
                emit_gath_c(pre, ct, 0)
            for g in range(n_grp2):
                wt = stream_tiles(g, ev_wsel, C * 128, wpool, "wsel_ev")
                gath = c_tiles.pop(g)
                emit_gath_c(g, gath, 1)
                if g + 2 < n_grp2:
                    ct = xpool.tile([128, GP * C * 256], BF16, tag="gx")
                    c_tiles[g + 2] = ct
                    emit_gath_c(g + 2, ct, 0)
                for r in range(GP):
                    b = g * GP + r
                    y_nm = spool.tile([128, 256], BF16, tag="y_nm")
                    for oh in range(2):
                        st = (pagg if oh == 0 else pagg2).tile(
                            [128, 128], F32, tag="agg" if oh == 0 else "st")
                        for j in range(C):
                            gp_, cp = cpos_f(j, r), cpos_f(j, r)
                            nc.tensor.matmul(
                                out=st[:],
                                lhsT=gath[:, gp_ * 256 + oh * 128:
                                          gp_ * 256 + oh * 128 + 128],
                                rhs=wt[:, cp * 128:(cp + 1) * 128],
                                start=(j == 0), stop=False)
                        osl = slice(oh * 128, (oh + 1) * 128)
                        nc.tensor.matmul(out=st[:], lhsT=W["w_rs"][0][:, osl],
                                         rhs=h2T[:, b * 128:(b + 1) * 128],
                                         start=False, stop=False)
                        nc.tensor.matmul(out=st[:], lhsT=W["w_rs"][1][:, osl],
                                         rhs=dT[:, b * 128:(b + 1) * 128],
                                         start=False, stop=False)
                        bias_mm(st[:], W["bias_c"][0][:, osl], last=True)
                        yT = spool.tile([128, 128], F32, tag="yT")
                        nc.scalar.activation(out=yT[:], in_=st[:],
                                             func=AF.Relu)
                        tp = pproj.tile([128, 128], F32, tag="pj1")
                        nc.tensor.transpose(out=tp[:], in_=yT[:],
                                            identity=ident[:])
                        nc.scalar.copy(y_nm[:, osl], tp[:])
                    nc.tensor.matmul(
                        out=pool_ps[:],
                        lhsT=ssel[:, b * B:(b + 1) * B],
                        rhs=y_nm[:],
                        start=(b == 0), stop=(b == n_blk - 1))

            def transpose_2(src_t, tag):
                ts = []
                for kh in range(2):
                    t_ps = pproj.tile([128, B], F32, tag="pj1")
                    nc.tensor.transpose(out=t_ps[:],
                                        in_=src_t[:, kh * 128:(kh + 1) * 128],
                                        identity=ident[:B, :B])
                    t_sb = spool.tile([128, B], F32, tag=f"{tag}_sb{kh}")
                    nc.vector.tensor_copy(t_sb[:], t_ps[:])
                    ts.append(t_sb)
                return ts


            def mlp(rhss, wname, bname, act, out_halves, tag):
                outs = []
                for o in range(out_halves):
                    osl = slice(o * 128, (o + 1) * 128)
                    ps = pagg.tile([128, B], F32, tag="agg")
                    for si, r in enumerate(rhss):
                        nc.tensor.matmul(out=ps[:], lhsT=W[wname][si][:, osl],
                                         rhs=r[:], start=(si == 0),
                                         stop=(si == len(rhss) - 1))
                    t = spool.tile([128, B], F32, tag=f"{tag}_sb{o}")
                    nc.scalar.activation(out=t[:], in_=ps[:], func=act,
                                         bias=W[bname][o][:, :1], scale=1.0)
                    outs.append(t)
                return outs

            # seq branch is independent of the graph -> emit before the AR
            seq = spool.tile([B, 256], F32, tag="seq")
            nc.sync.dma_start(out=seq[:], in_=seq_in[:])
            seqT = transpose_2(seq, "seqT")
            s1T = mlp(seqT, "fc1_W", "fc1_b", AF.Relu, 2, "s1")
            sT = mlp(s1T, "fc2_W", "fc2_b", AF.Relu, 1, "s2")

            pooled = spool.tile([B, 256], F32, tag="pooled")
            nc.vector.tensor_copy(pooled[:], pool_ps[:])
            nc.sync.dma_start(out=ar_in[:], in_=pooled[:])
            nc.gpsimd.collective_compute(
                "AllReduce", OP.add, replica_groups=[list(range(NC))],
                ins=[ar_in.opt()], outs=[ar_out.opt()])

            # ---- head (replicated on every core) --------------------------
            emb = spool.tile([B, 256], F32, tag="emb")
            nc.sync.dma_start(out=emb[:], in_=ar_out[:])
            # (seq branch was hoisted before the AllReduce below)


            embT = transpose_2(emb, "embT")


            hT = mlp(embT + sT, "fcc_W", "fcc_b", AF.Relu, 2, "hc")

            lg_ps = pagg2.tile([B, 16], F32, tag="st")
            for o in range(2):
                nc.tensor.matmul(out=lg_ps[:], lhsT=hT[o][:],
                                 rhs=W["cls_W"][o][:],
                                 start=(o == 0), stop=(o == 1))
            logits = spool.tile([B, 16], F32, tag="logits")
            nc.vector.tensor_tensor(out=logits[:], in0=lg_ps[:],
                                    in1=W["cls_b_rep"][0][:], op=OP.add)
            rmax = spool.tile([B, 1], F32, tag="rmax")
            nc.vector.tensor_reduce(out=rmax[:], in_=logits[:],
                                    axis=mybir.AxisListType.X, op=OP.max)
            tshift = spool.tile([B, 16], F32, tag="tshift")
            nc.vector.tensor_scalar(out=tshift[:], in0=logits[:],
                                    scalar1=rmax[:, :1], scalar2=None,
                                    op0=OP.subtract)
            ex = spool.tile([B, 16], F32, tag="ex")
            nc.scalar.activation(out=ex[:], in_=tshift[:], func=AF.Exp)
            esum = spool.tile([B, 1], F32, tag="esum")
            nc.vector.tensor_reduce(out=esum[:], in_=ex[:],
                                    axis=mybir.AxisListType.X, op=OP.add)
            lsum = spool.tile([B, 1], F32, tag="lsum")
            nc.scalar.activation(out=lsum[:], in_=esum[:], func=AF.Ln)
            res = spool.tile([B, 16], F32, tag="res")
            nc.vector.tensor_scalar(out=res[:], in0=tshift[:],
                                    scalar1=lsum[:, :1], scalar2=None,
                                    op0=OP.subtract)
            nc.sync.dma_start(out=out_ext[:], in_=res[:])

    nc.compile()
    return nc


# --------------------------------------------------------------------------
# Host orchestration
# --------------------------------------------------------------------------

def make_in_maps(inputs, cfg):
    import ml_dtypes
    BF = ml_dtypes.bfloat16

    x = np.asarray(inputs["x"], np.float32)
    if np.any(x == -1.0):
        x = x * (x != -1.0)
    dur_x = np.asarray(inputs["dur_x"], np.float32)
    batch = np.asarray(inputs["batch"], np.int64)
    B = cfg["B"]
    n_blk = cfg["SHARD_PAD"] // 128

    x_pad = _pad_nodes(x, cfg).astype(BF)
    dur_pad = _pad_nodes(dur_x, cfg).astype(BF)

    ev_planes, CA, CB = prep_edges(inputs["edge_index"], inputs["edge_attr"],
                                   x_pad, 128, cfg)
    du_planes, CDA, CDB = prep_edges(inputs["dur_edge_index"],
                                     inputs["dur_edge_attr"], dur_pad, 64, cfg,
                                     min_ca=CA, min_cb=CB)
    if CDA > CA or CDB > CB:
        # shared wsel stream tile needs identical chunk geometry
        ev_planes, CA, CB = prep_edges(
            inputs["edge_index"], inputs["edge_attr"], x_pad, 128, cfg,
            min_ca=CDA, min_cb=CDB)
    assert (CA, CB) == (CDA, CDB)

    cnt = np.bincount(batch, minlength=B).astype(np.float32)
    inv_cnt = 1.0 / np.maximum(cnt, 1.0)

    def row(v):
        return np.asarray(v, np.float32).reshape(1, -1)

    def col(v):
        return np.asarray(v, np.float32).reshape(-1, 1)

    w_rs = (np.asarray(inputs["c1_Wroot"], np.float32)
            + np.asarray(inputs["skip_W"], np.float32))
    bias_c = (np.asarray(inputs["c1_br"], np.float32)
              + np.asarray(inputs["skip_b"], np.float32))

    def asbf(v):
        return np.ascontiguousarray(np.asarray(v, np.float32)).astype(BF)

    weights = dict(
        g1_Wr=asbf(inputs["g1_Wr"]), g1_Wroot=asbf(inputs["g1_Wroot"]),
        g1_br=asbf(row(inputs["g1_br"])),
        g2_Wr=asbf(inputs["g2_Wr"]), g2_Wroot=asbf(inputs["g2_Wroot"]),
        g2_br=asbf(row(inputs["g2_br"])),
        d1_Wr=asbf(inputs["d1_Wr"]), d1_Wroot=asbf(inputs["d1_Wroot"]),
        d1_br=asbf(row(inputs["d1_br"])),
        c1_Wr=asbf(inputs["c1_Wr"]), w_rs=asbf(w_rs),
        bias_c=asbf(row(bias_c)),
        fc1_W=np.asarray(inputs["fc1_W"], np.float32),
        fc1_b=col(inputs["fc1_b"]),
        fc2_W=np.asarray(inputs["fc2_W"], np.float32),
        fc2_b=col(inputs["fc2_b"]),
        fcc_W=np.asarray(inputs["fcc_W"], np.float32),
        fcc_b=col(inputs["fcc_b"]),
        cls_W=np.asarray(inputs["cls_W"], np.float32),
        cls_b_rep=np.tile(np.asarray(inputs["cls_b"], np.float32)[None, :],
                          (B, 1)),
        seq_features=np.asarray(inputs["seq_features"], np.float32),
    )
    weights = {k: np.ascontiguousarray(v) for k, v in weights.items()}

    in_maps = []
    for k in range(NC):
        sp = cfg["SHARD_PAD"]
        sh = cfg["SHARD"]
        # feature-major local node features
        xT = np.ascontiguousarray(x_pad[k * sp:(k + 1) * sp].T)
        dxT = np.ascontiguousarray(dur_pad[k * sp:(k + 1) * sp].T)
        # pooling one-hot with 1/cnt baked in: [128, n_blk*B]
        bl = np.full(sp, -1, np.int64)
        bl[:sh] = batch[k * sh:(k + 1) * sh]
        ssel = np.zeros((128, n_blk * B), np.float32)
        pos = np.arange(sp)
        valid = bl >= 0
        ssel[pos[valid] % 128,
             (pos[valid] // 128) * B + bl[valid]] = inv_cnt[bl[valid]]
        ssel = ssel.astype(BF)
        m = dict(
            ev_wsel=ev_planes[k]["wsel"], ev_xg=ev_planes[k]["xg"],
            ev_idx_a=ev_planes[k]["idx_a"], ev_idx_b=ev_planes[k]["idx_b"],
            du_wsel=du_planes[k]["wsel"], du_xg=du_planes[k]["xg"],
            xT=xT, dxT=dxT, ssel=ssel,
            **weights,
        )
        in_maps.append(m)
    return in_maps, (CA, CB, CDA, CDB)


_LAST_RESULT = None


def kernel(**inputs) -> np.ndarray:
    global _LAST_RESULT
    cfg = dict(REAL)
    cfg["N"] = inputs["x"].shape[0]
    cfg["B"] = inputs["seq_features"].shape[0]
    in_maps, (CA, CB, CDA, CDB) = make_in_maps(inputs, cfg)
    nc = build_program(cfg, CA, CB, CDA, CDB)
    from concourse.bass_utils import run_bass_kernel_spmd
    res = run_bass_kernel_spmd(nc, in_maps, list(range(NC)))
    _LAST_RESULT = res
    return np.asarray(res.results[0]["out"], np.float32)
